# revision 17
# baseline (speedup 1.0000x reference)
"""Fused multi-head cross-attention for Trainium2, SPMD over 8 NeuronCores.

Problem: out = softmax(q @ k^T / sqrt(D) + attn_bias + pad_mask) @ v
  q: (B=4, Sq=2048, H=16, D=128) f32, kv: (B, Sk=2048, 2, H, D) f32,
  attn_bias: (B, Sk) f32, key_padding_mask: (B, Sk) bool -> out (B, Sq, H, D) f32

Sharding: 64 (b, h) slices; core k owns batch k//2, heads (k%2)*8..+8.

v2 design (engine-balanced around the ACT-engine exp roofline):
  - Q^T, K^T are pre-transposed on the host (D-major), so device loads are
    linear DMAs (no XBAR transpose).  S^T = K @ Q^T is computed in 512-wide
    PSUM blocks; ACT exponentiates (exp(scale*s), bias NOT fused here).
  - The per-key bias factor e^b/16 is folded into pp in place by DVE
    tensor_scalar ops (4x mode), so BOTH the PV numerator and the softmax
    denominator see it; the 1/16 prescale cancels in the normalize and keeps
    the fp16 tree below overflow.
  - PV runs TRANSPOSED (out^T = sum_c V_c^T @ P'_c) so all PE matmuls have
    512-wide moving operands (256 short 129-wide matmuls in v1 ran at the
    ~81ns/MM production floor and made PE the bottleneck at ~276us/core).
  - The softmax denominator comes from an fp16 pairwise tree over pp chunks
    (DVE 2x) + a 1-column ones matmul per 512-q block; 1/l is computed by
    reciprocal_approx_fast and broadcast across partitions with a K=1 f32
    matmul into PSUM; DVE multiplies out^T by it.
  - PSUM: 2x3 banks S^T double buffer + 1 bank PV accum + 1 shared bank for
    the l row / broadcast = exactly 8.
"""

import sys

if "/opt/trn_rl_repo" not in sys.path:
    sys.path.insert(0, "/opt/trn_rl_repo")

import numpy as np
import ml_dtypes

B, SQ, SK, H, D = 4, 2048, 2048, 16, 128
NCORES = 8
NSL = H * B // NCORES  # 8 head-slices per core
CK = SK // 128  # 16 sk chunks
NQB = SQ // 512  # 4 q blocks of 512
SCALE = float(1.0 / np.sqrt(np.float32(D)))
EBSCL = 1.0 / 16.0  # prescale on e^bias; cancels in normalize, guards fp16

_CACHE = {}


def _build_nc(nrep=1, lead=4, tree_lag=3, pss_bufs=2, pv_bufs=2):
    """nrep > 1 repeats the whole per-core computation (same inputs/outputs)
    back-to-back; used only for wall-clock timing (device work >> RPC cost)."""
    import concourse.bacc as bacc
    import concourse.tile as tile
    import concourse.mybir as mybir

    f32 = mybir.dt.float32
    bf16 = mybir.dt.bfloat16
    f16 = mybir.dt.float16
    mul_op = mybir.AluOpType.mult
    add_op = mybir.AluOpType.add

    nc = bacc.Bacc("TRN2", target_bir_lowering=False, debug=False)
    qd = nc.dram_tensor("qT", [NSL, D, SQ], bf16, kind="ExternalInput").ap()
    kd = nc.dram_tensor("kT", [NSL, D, SK], bf16, kind="ExternalInput").ap()
    vd = nc.dram_tensor("vb", [NSL, SK, D], bf16, kind="ExternalInput").ap()
    # e^(attn_bias + mask)/16 per key, laid out (sk%128, chunk)
    bd = nc.dram_tensor("ebT", [128, CK], f32, kind="ExternalInput").ap()
    od = nc.dram_tensor("out", [NSL, D, SQ], f32, kind="ExternalOutput").ap()

    GRP = 2  # blocks per S^T group; chunk-pure so the ACT bias is uniform
    NBLK = CK * NQB  # 64 S^T blocks per slice
    groups = [(b0, GRP) for b0 in range(0, NBLK, GRP)]

    # group index after which chunk c is fully exponentiated
    def grp_done(c):
        return 2 * c + 1

    with tile.TileContext(nc) as tc:
        with (
            tc.tile_pool(name="qt", bufs=2) as qt_pool,
            tc.tile_pool(name="kt", bufs=2) as kt_pool,
            tc.tile_pool(name="vp", bufs=3) as vp_pool,
            tc.tile_pool(name="pp", bufs=2) as pp_pool,
            tc.tile_pool(name="swl", bufs=1) as swl_pool,
            tc.tile_pool(name="swsum", bufs=2) as swsum_pool,
            tc.tile_pool(name="bias", bufs=1) as bias_pool,
            tc.tile_pool(name="ot", bufs=2) as ot_pool,
            tc.tile_pool(name="rl", bufs=1) as rl_pool,
            tc.tile_pool(name="rlb", bufs=1) as rlb_pool,
            tc.tile_pool(name="psS", bufs=pss_bufs, space="PSUM") as psS_pool,  # 2 banks
            tc.tile_pool(name="psV", bufs=pv_bufs, space="PSUM") as psV_pool,  # 1 bank
            tc.tile_pool(name="psL", bufs=1, space="PSUM") as psL_pool,  # 1 bank
        ):
            bias_sb = bias_pool.tile([128, CK], f32)
            nc.sync.dma_start(bias_sb[:], bd[:])
            ones_col = bias_pool.tile([128, 1], f16)
            nc.vector.memset(ones_col[:], 1.0)
            ones_row = bias_pool.tile([1, 128], f16)
            nc.vector.memset(ones_row[:], 1.0)
            # pairwise-tree scratch: 9 slots of [128, SQ] f16
            swl = swl_pool.tile([128, 9, SQ], f16)

            slice_tiles = {}

            def load_slice(s):
                qt_t = qt_pool.tile([128, SQ], bf16)
                nc.sync.dma_start(qt_t[:], qd[s])
                kt_t = kt_pool.tile([128, SK], bf16)
                nc.sync.dma_start(kt_t[:], kd[s])
                vp_t = vp_pool.tile([128, CK, D], bf16)
                nc.sync.dma_start(
                    vp_t[:], vd[s].rearrange("(c p) d -> p c d", p=128)
                )
                pp_t = pp_pool.tile([128, CK, SQ], bf16)
                sw_t = swsum_pool.tile([128, SQ], f16)
                slice_tiles[s] = (qt_t, kt_t, vp_t, pp_t, sw_t)

            def s_group(s, gi):
                b0, nblk = groups[gi]
                qt_t, kt_t, _, pp_t, _ = slice_tiles[s]
                c0 = b0 // NQB
                ps = psS_pool.tile([128, GRP * 512], f32, name=f"ps{gi}", tag="ps")
                for j in range(nblk):
                    b = b0 + j
                    c, qcol = divmod(b, NQB)
                    nc.tensor.matmul(
                        ps[:, j * 512 : (j + 1) * 512],
                        lhsT=kt_t[:, c * 128 : (c + 1) * 128],
                        rhs=qt_t[:, qcol * 512 : (qcol + 1) * 512],
                        start=True,
                        stop=True,
                    )
                pp_flat = pp_t.rearrange("p c q -> p (c q)")
                nc.scalar.activation(
                    pp_flat[:, b0 * 512 : (b0 + nblk) * 512],
                    ps[:, 0 : nblk * 512],
                    mybir.ActivationFunctionType.Exp,
                    bias=bias_sb[:, c0 : c0 + 1],
                    scale=SCALE,
                )

            # DVE work tied to S groups: fold e^b/16 into pp in place; build
            # the pairwise fp16 tree toward swsum.  Returns list keyed by
            # group index -> list of callables.
            def dve_plan(s):
                _, _, _, pp_t, sw_t = slice_tiles[s]
                plan = {gi: [] for gi in range(len(groups))}

                def tt(out_ap, a_ap, b_ap):
                    def f():
                        nc.vector.tensor_tensor(out_ap, a_ap, b_ap, add_op)
                    return f

                n_g = len(groups)

                def lag(gi):
                    return min(gi + tree_lag, n_g - 1)

                # L0: 8 pair sums of pp chunks -> swl slots 0..7
                for j in range(8):
                    plan[lag(grp_done(2 * j + 1))].append(
                        tt(swl[:, j, :], pp_t[:, 2 * j, :], pp_t[:, 2 * j + 1, :])
                    )
                g_l1 = [lag(grp_done(3)), lag(grp_done(7)), lag(grp_done(11)),
                        lag(grp_done(15))]
                # L1: slots (0,1)->8, (2,3)->0', reuse via distinct slots
                plan[g_l1[0]].append(tt(swl[:, 8, :], swl[:, 0, :], swl[:, 1, :]))
                plan[g_l1[1]].append(tt(swl[:, 0, :], swl[:, 2, :], swl[:, 3, :]))
                plan[g_l1[2]].append(tt(swl[:, 1, :], swl[:, 4, :], swl[:, 5, :]))
                plan[g_l1[3]].append(tt(swl[:, 2, :], swl[:, 6, :], swl[:, 7, :]))
                # L2 + L3 -> swsum
                plan[g_l1[3]].append(tt(swl[:, 3, :], swl[:, 8, :], swl[:, 0, :]))
                plan[g_l1[3]].append(tt(swl[:, 4, :], swl[:, 1, :], swl[:, 2, :]))
                plan[g_l1[3]].append(tt(sw_t[:], swl[:, 3, :], swl[:, 4, :]))
                return plan

            # Output phase for slice s (runs one step later): per 512-q block:
            # l row matmul, 1/l, partition-broadcast matmul, 16 PV^T matmuls,
            # normalize, store.
            def out_phase_items(s):
                _, _, vp_t, pp_t, sw_t = slice_tiles[s]
                items = []
                state = {}

                def psl(j):
                    def f():
                        ps_l = psL_pool.tile([1, 512], f32, name=f"psl{j}", tag="psl")
                        state["psl"] = ps_l
                        nc.tensor.matmul(
                            ps_l[:],
                            lhsT=ones_col[:],
                            rhs=sw_t[:, j * 512 : (j + 1) * 512],
                            start=True,
                            stop=True,
                        )
                    return f

                def rb(j):
                    def f():
                        from concourse.dve_ops import (
                            RECIP_APPROX_FAST_CONSTS,
                            RECIPROCAL_APPROX_FAST,
                        )
                        rc = RECIP_APPROX_FAST_CONSTS
                        rl_h = rl_pool.tile([1, 512], f16, name=f"rlh{j}", tag="rl")
                        nc.vector._custom_dve(
                            RECIPROCAL_APPROX_FAST,
                            out=rl_h[:],
                            in0=state["psl"][:],
                            s0=rc["s0"],
                            s1=rc["s1"],
                            imm2=rc["imm2"],
                        )
                        rlb = psL_pool.tile([128, 512], f32, name=f"rlb{j}", tag="psl")
                        nc.tensor.matmul(
                            rlb[:], lhsT=ones_row[:], rhs=rl_h[:],
                            start=True, stop=True,
                        )
                        rlb_sb = rlb_pool.tile(
                            [128, 512], f32, name=f"rlbs{j}", tag="rlbs"
                        )
                        state["rlb"] = rlb_sb
                        nc.vector.tensor_copy(rlb_sb[:], rlb[:])
                    return f

                def pv(j, c0, c1):
                    def f():
                        if c0 == 0:
                            state["pv"] = psV_pool.tile([128, 512], f32, name=f"pv{j}", tag="pv")
                        po = state["pv"]
                        for c in range(c0, c1):
                            nc.tensor.matmul(
                                po[:],
                                lhsT=vp_t[:, c, :],
                                rhs=pp_t[:, c, j * 512 : (j + 1) * 512],
                                start=(c == 0),
                                stop=(c == CK - 1),
                            )
                    return f

                def fin(j):
                    def f():
                        ot = ot_pool.tile([128, 512], f32, name=f"ot{j}", tag="ot")
                        nc.vector.tensor_tensor(
                            ot[:], state["pv"][:], state["rlb"][:], mul_op
                        )
                        nc.sync.dma_start(od[s, :, j * 512 : (j + 1) * 512], ot[:])
                    return f

                for j in range(NQB):
                    items.append(psl(j))
                    items.append(pv(j, 0, 5))
                    items.append(rb(j))
                    items.append(pv(j, 5, 10))
                    items.append(pv(j, 10, CK))
                    items.append(fin(j))
                return items

            NV = NSL * nrep  # total virtual slices

            def emit_step(v):
                s = v % NSL
                do_s = v < NV
                if v + 1 < NV:
                    load_slice((v + 1) % NSL)
                oq = out_phase_items((v - 1) % NSL) if v > 0 else []
                if not do_s:
                    for f in oq:
                        f()
                    return
                plan = dve_plan(s)
                n_g = len(groups)
                oi = 0
                acc = 0.0
                ratio = len(oq) / max(1, n_g - lead)
                for gi in range(n_g):
                    s_group(s, gi)
                    for f in plan[gi]:
                        f()
                    if gi >= lead:
                        acc += ratio
                        while acc >= 1.0 and oi < len(oq):
                            oq[oi]()
                            oi += 1
                            acc -= 1.0
                while oi < len(oq):
                    oq[oi]()
                    oi += 1

            load_slice(0)
            for v in range(NV + 1):
                emit_step(v)

    nc.compile()
    return nc


def _get_nc():
    if "nc" not in _CACHE:
        _CACHE["nc"] = _build_nc()
    return _CACHE["nc"]


def _make_in_maps(q, kv, attn_bias, key_padding_mask):
    q = np.asarray(q)
    kv = np.asarray(kv)
    attn_bias = np.asarray(attn_bias, dtype=np.float32)
    key_padding_mask = np.asarray(key_padding_mask)

    biasp = attn_bias + np.where(key_padding_mask, 0.0, -1e30).astype(np.float32)
    ebias = (biasp + np.log(EBSCL)).astype(np.float32)  # additive; masked -> -inf
    bf16 = ml_dtypes.bfloat16

    in_maps = []
    for core in range(NCORES):
        b = core // (NCORES // B)
        h0 = (core % (NCORES // B)) * NSL
        # (Sq, NSL, D) -> (NSL, D, Sq) pre-transposed for D-major loads
        qb = np.ascontiguousarray(
            q[b, :, h0 : h0 + NSL, :].transpose(1, 2, 0)
        ).astype(bf16)
        kb = np.ascontiguousarray(
            kv[b, :, 0, h0 : h0 + NSL, :].transpose(1, 2, 0)
        ).astype(bf16)
        vb = np.ascontiguousarray(
            kv[b, :, 1, h0 : h0 + NSL, :].transpose(1, 0, 2)
        ).astype(bf16)
        ebT = np.ascontiguousarray(ebias[b].reshape(CK, 128).T.astype(np.float32))
        in_maps.append({"qT": qb, "kT": kb, "vb": vb, "ebT": ebT})
    return in_maps


def _gather(results):
    out = np.empty((B, SQ, H, D), dtype=np.float32)
    for core in range(NCORES):
        b = core // (NCORES // B)
        h0 = (core % (NCORES // B)) * NSL
        # device out is (NSL, D, SQ) -> (SQ, NSL, D)
        out[b, :, h0 : h0 + NSL, :] = results[core]["out"].transpose(2, 0, 1)
    return out


def kernel(q, kv, attn_bias, key_padding_mask):
    from concourse.bass_utils import run_bass_kernel_spmd

    nc = _get_nc()
    in_maps = _make_in_maps(q, kv, attn_bias, key_padding_mask)
    res = run_bass_kernel_spmd(nc, in_maps, list(range(NCORES)))
    return _gather(res.results)


# revision 19
# speedup vs baseline: 1.0146x; 1.0146x over previous
"""Fused multi-head cross-attention for Trainium2, SPMD over 8 NeuronCores.

Problem: out = softmax(q @ k^T / sqrt(D) + attn_bias + pad_mask) @ v
  q: (B=4, Sq=2048, H=16, D=128) f32, kv: (B, Sk=2048, 2, H, D) f32,
  attn_bias: (B, Sk) f32, key_padding_mask: (B, Sk) bool -> out (B, Sq, H, D) f32

Sharding: 64 (b, h) slices; core k owns batch k//2, heads (k%2)*8..+8.

v3 design (engine-balanced around the ACT-engine exp roofline, ~291us/rep
HW vs 318.7us baseline; ACT busy ~266us is the modeled floor):
  - Q^T, K^T are pre-transposed on the host (D-major), so device loads are
    linear DMAs (no XBAR transpose).  S^T = K @ Q^T is computed in 512-wide
    PSUM blocks, chunk-pure pairs of banks, so the per-key attn bias
    (+ mask - ln16) fuses into the ACT exponential's per-partition bias
    slot at zero extra engine cost: pp = exp(scale*s + b)/16.  The 1/16
    prescale cancels in the normalize and keeps the fp16 tree < 65504.
  - PV runs TRANSPOSED (out^T = sum_c V_c^T @ P'_c per 512-q block) so all
    PE matmuls have 512-wide moving operands; v1's 256 short 129-wide PV
    matmuls ran at the ~81ns/MM LDWEIGHTS floor and made PE the bottleneck.
  - The softmax denominator: fp16 pairwise tree over pp chunks (DVE 2x),
    then per 512-q block a 1-column ones matmul gives l = sum_k P', a
    custom-DVE reciprocal (f16 out) gives 1/l, a K=1 fp16 ones matmul
    broadcasts it across partitions into PSUM, DVE copies it to SBUF
    (DVE cannot read 2 PSUM operands) and multiplies out^T by it.
  - NO GPSIMD: its tensor ops cost ~25us per [128,2048] op on HW.
  - PSUM: 3x2 banks S^T triple buffer + 1 bank PV accum + 1 shared bank
    psl/rlb = exactly 8.  Triple-buffered S^T is load-bearing: bufs=2
    measured 318.7us vs 291.2us.
"""

import sys

if "/opt/trn_rl_repo" not in sys.path:
    sys.path.insert(0, "/opt/trn_rl_repo")

import numpy as np
import ml_dtypes

B, SQ, SK, H, D = 4, 2048, 2048, 16, 128
NCORES = 8
NSL = H * B // NCORES  # 8 head-slices per core
CK = SK // 128  # 16 sk chunks
NQB = SQ // 512  # 4 q blocks of 512
SCALE = float(1.0 / np.sqrt(np.float32(D)))
EBSCL = 1.0 / 16.0  # prescale on e^bias; cancels in normalize, guards fp16

_CACHE = {}


def _build_nc(nrep=1, lead=4, tree_lag=3, pss_bufs=3, pv_bufs=1):
    """nrep > 1 repeats the whole per-core computation (same inputs/outputs)
    back-to-back; used only for wall-clock timing (device work >> RPC cost)."""
    import concourse.bacc as bacc
    import concourse.tile as tile
    import concourse.mybir as mybir

    f32 = mybir.dt.float32
    bf16 = mybir.dt.bfloat16
    f16 = mybir.dt.float16
    mul_op = mybir.AluOpType.mult
    add_op = mybir.AluOpType.add

    nc = bacc.Bacc("TRN2", target_bir_lowering=False, debug=False)
    qd = nc.dram_tensor("qT", [NSL, D, SQ], bf16, kind="ExternalInput").ap()
    kd = nc.dram_tensor("kT", [NSL, D, SK], bf16, kind="ExternalInput").ap()
    vd = nc.dram_tensor("vb", [NSL, SK, D], bf16, kind="ExternalInput").ap()
    # additive attn_bias + mask - ln16 per key, laid out (sk%128, chunk)
    bd = nc.dram_tensor("ebT", [128, CK], f32, kind="ExternalInput").ap()
    od = nc.dram_tensor("out", [NSL, D, SQ], f32, kind="ExternalOutput").ap()

    GRP = 2  # blocks per S^T group; chunk-pure so the ACT bias is uniform
    NBLK = CK * NQB  # 64 S^T blocks per slice
    groups = [(b0, GRP) for b0 in range(0, NBLK, GRP)]

    # group index after which chunk c is fully exponentiated
    def grp_done(c):
        return 2 * c + 1

    with tile.TileContext(nc) as tc:
        with (
            tc.tile_pool(name="qt", bufs=2) as qt_pool,
            tc.tile_pool(name="kt", bufs=2) as kt_pool,
            tc.tile_pool(name="vp", bufs=3) as vp_pool,
            tc.tile_pool(name="pp", bufs=2) as pp_pool,
            tc.tile_pool(name="swl", bufs=1) as swl_pool,
            tc.tile_pool(name="swsum", bufs=2) as swsum_pool,
            tc.tile_pool(name="bias", bufs=1) as bias_pool,
            tc.tile_pool(name="ot", bufs=2) as ot_pool,
            tc.tile_pool(name="rl", bufs=1) as rl_pool,
            tc.tile_pool(name="rlb", bufs=1) as rlb_pool,
            tc.tile_pool(name="psS", bufs=pss_bufs, space="PSUM") as psS_pool,  # 2 banks
            tc.tile_pool(name="psV", bufs=pv_bufs, space="PSUM") as psV_pool,  # 1 bank
            tc.tile_pool(name="psL", bufs=1, space="PSUM") as psL_pool,  # 1 bank
        ):
            bias_sb = bias_pool.tile([128, CK], f32)
            nc.sync.dma_start(bias_sb[:], bd[:])
            ones_col = bias_pool.tile([128, 1], f16)
            nc.vector.memset(ones_col[:], 1.0)
            ones_row = bias_pool.tile([1, 128], f16)
            nc.vector.memset(ones_row[:], 1.0)
            # pairwise-tree scratch: 9 slots of [128, SQ] f16
            swl = swl_pool.tile([128, 9, SQ], f16)

            slice_tiles = {}

            def load_slice(s):
                qt_t = qt_pool.tile([128, SQ], bf16)
                nc.sync.dma_start(qt_t[:], qd[s])
                kt_t = kt_pool.tile([128, SK], bf16)
                nc.sync.dma_start(kt_t[:], kd[s])
                vp_t = vp_pool.tile([128, CK, D], bf16)
                nc.sync.dma_start(
                    vp_t[:], vd[s].rearrange("(c p) d -> p c d", p=128)
                )
                pp_t = pp_pool.tile([128, CK, SQ], bf16)
                sw_t = swsum_pool.tile([128, SQ], f16)
                slice_tiles[s] = (qt_t, kt_t, vp_t, pp_t, sw_t)

            def s_group(s, gi):
                b0, nblk = groups[gi]
                qt_t, kt_t, _, pp_t, _ = slice_tiles[s]
                c0 = b0 // NQB
                ps = psS_pool.tile([128, GRP * 512], f32, name=f"ps{gi}", tag="ps")
                for j in range(nblk):
                    b = b0 + j
                    c, qcol = divmod(b, NQB)
                    nc.tensor.matmul(
                        ps[:, j * 512 : (j + 1) * 512],
                        lhsT=kt_t[:, c * 128 : (c + 1) * 128],
                        rhs=qt_t[:, qcol * 512 : (qcol + 1) * 512],
                        start=True,
                        stop=True,
                    )
                pp_flat = pp_t.rearrange("p c q -> p (c q)")
                nc.scalar.activation(
                    pp_flat[:, b0 * 512 : (b0 + nblk) * 512],
                    ps[:, 0 : nblk * 512],
                    mybir.ActivationFunctionType.Exp,
                    bias=bias_sb[:, c0 : c0 + 1],
                    scale=SCALE,
                )

            # DVE work tied to S groups: the pairwise fp16 tree summing pp
            # chunks toward swsum (the softmax denominator row).  Returns
            # a dict keyed by group index -> list of emission callables.
            def dve_plan(s):
                _, _, _, pp_t, sw_t = slice_tiles[s]
                plan = {gi: [] for gi in range(len(groups))}

                def tt(out_ap, a_ap, b_ap):
                    def f():
                        nc.vector.tensor_tensor(out_ap, a_ap, b_ap, add_op)
                    return f

                n_g = len(groups)

                def lag(gi):
                    return min(gi + tree_lag, n_g - 1)

                # L0: 8 pair sums of pp chunks -> swl slots 0..7
                for j in range(8):
                    plan[lag(grp_done(2 * j + 1))].append(
                        tt(swl[:, j, :], pp_t[:, 2 * j, :], pp_t[:, 2 * j + 1, :])
                    )
                g_l1 = [lag(grp_done(3)), lag(grp_done(7)), lag(grp_done(11)),
                        lag(grp_done(15))]
                # L1: slots (0,1)->8, (2,3)->0', reuse via distinct slots
                plan[g_l1[0]].append(tt(swl[:, 8, :], swl[:, 0, :], swl[:, 1, :]))
                plan[g_l1[1]].append(tt(swl[:, 0, :], swl[:, 2, :], swl[:, 3, :]))
                plan[g_l1[2]].append(tt(swl[:, 1, :], swl[:, 4, :], swl[:, 5, :]))
                plan[g_l1[3]].append(tt(swl[:, 2, :], swl[:, 6, :], swl[:, 7, :]))
                # L2 + L3 -> swsum
                plan[g_l1[3]].append(tt(swl[:, 3, :], swl[:, 8, :], swl[:, 0, :]))
                plan[g_l1[3]].append(tt(swl[:, 4, :], swl[:, 1, :], swl[:, 2, :]))
                plan[g_l1[3]].append(tt(sw_t[:], swl[:, 3, :], swl[:, 4, :]))
                return plan

            # Output phase for slice s (runs one step later): per 512-q block:
            # l row matmul, 1/l, partition-broadcast matmul, 16 PV^T matmuls,
            # normalize, store.
            def out_phase_items(s):
                _, _, vp_t, pp_t, sw_t = slice_tiles[s]
                items = []
                state = {}

                def psl(j):
                    def f():
                        ps_l = psL_pool.tile([1, 512], f32, name=f"psl{j}", tag="psl")
                        state["psl"] = ps_l
                        nc.tensor.matmul(
                            ps_l[:],
                            lhsT=ones_col[:],
                            rhs=sw_t[:, j * 512 : (j + 1) * 512],
                            start=True,
                            stop=True,
                        )
                    return f

                def rb(j):
                    def f():
                        from concourse.dve_ops import (
                            RECIP_APPROX_FAST_CONSTS,
                            RECIPROCAL_APPROX_FAST,
                        )
                        rc = RECIP_APPROX_FAST_CONSTS
                        rl_h = rl_pool.tile([1, 512], f16, name=f"rlh{j}", tag="rl")
                        nc.vector._custom_dve(
                            RECIPROCAL_APPROX_FAST,
                            out=rl_h[:],
                            in0=state["psl"][:],
                            s0=rc["s0"],
                            s1=rc["s1"],
                            imm2=rc["imm2"],
                        )
                        rlb = psL_pool.tile([128, 512], f32, name=f"rlb{j}", tag="psl")
                        nc.tensor.matmul(
                            rlb[:], lhsT=ones_row[:], rhs=rl_h[:],
                            start=True, stop=True,
                        )
                        rlb_sb = rlb_pool.tile(
                            [128, 512], f32, name=f"rlbs{j}", tag="rlbs"
                        )
                        state["rlb"] = rlb_sb
                        nc.vector.tensor_copy(rlb_sb[:], rlb[:])
                    return f

                def pv(j, c0, c1):
                    def f():
                        if c0 == 0:
                            state["pv"] = psV_pool.tile([128, 512], f32, name=f"pv{j}", tag="pv")
                        po = state["pv"]
                        for c in range(c0, c1):
                            nc.tensor.matmul(
                                po[:],
                                lhsT=vp_t[:, c, :],
                                rhs=pp_t[:, c, j * 512 : (j + 1) * 512],
                                start=(c == 0),
                                stop=(c == CK - 1),
                            )
                    return f

                def fin(j):
                    def f():
                        ot = ot_pool.tile([128, 512], f32, name=f"ot{j}", tag="ot")
                        nc.vector.tensor_tensor(
                            ot[:], state["pv"][:], state["rlb"][:], mul_op
                        )
                        nc.sync.dma_start(od[s, :, j * 512 : (j + 1) * 512], ot[:])
                    return f

                for j in range(NQB):
                    items.append(psl(j))
                    items.append(pv(j, 0, 5))
                    items.append(rb(j))
                    items.append(pv(j, 5, 10))
                    items.append(pv(j, 10, CK))
                    items.append(fin(j))
                return items

            NV = NSL * nrep  # total virtual slices

            def emit_step(v):
                s = v % NSL
                do_s = v < NV
                if v + 1 < NV:
                    load_slice((v + 1) % NSL)
                oq = out_phase_items((v - 1) % NSL) if v > 0 else []
                if not do_s:
                    for f in oq:
                        f()
                    return
                plan = dve_plan(s)
                n_g = len(groups)
                oi = 0
                acc = 0.0
                ratio = len(oq) / max(1, n_g - lead)
                for gi in range(n_g):
                    s_group(s, gi)
                    for f in plan[gi]:
                        f()
                    if gi >= lead:
                        acc += ratio
                        while acc >= 1.0 and oi < len(oq):
                            oq[oi]()
                            oi += 1
                            acc -= 1.0
                while oi < len(oq):
                    oq[oi]()
                    oi += 1

            load_slice(0)
            for v in range(NV + 1):
                emit_step(v)

    nc.compile()
    return nc


def _get_nc():
    if "nc" not in _CACHE:
        _CACHE["nc"] = _build_nc()
    return _CACHE["nc"]


def _make_in_maps(q, kv, attn_bias, key_padding_mask):
    q = np.asarray(q)
    kv = np.asarray(kv)
    attn_bias = np.asarray(attn_bias, dtype=np.float32)
    key_padding_mask = np.asarray(key_padding_mask)

    biasp = attn_bias + np.where(key_padding_mask, 0.0, -1e30).astype(np.float32)
    ebias = (biasp + np.log(EBSCL)).astype(np.float32)  # additive; masked -> -inf
    bf16 = ml_dtypes.bfloat16

    in_maps = []
    for core in range(NCORES):
        b = core // (NCORES // B)
        h0 = (core % (NCORES // B)) * NSL
        # (Sq, NSL, D) -> (NSL, D, Sq) pre-transposed for D-major loads
        qb = np.ascontiguousarray(
            q[b, :, h0 : h0 + NSL, :].transpose(1, 2, 0)
        ).astype(bf16)
        kb = np.ascontiguousarray(
            kv[b, :, 0, h0 : h0 + NSL, :].transpose(1, 2, 0)
        ).astype(bf16)
        vb = np.ascontiguousarray(
            kv[b, :, 1, h0 : h0 + NSL, :].transpose(1, 0, 2)
        ).astype(bf16)
        ebT = np.ascontiguousarray(ebias[b].reshape(CK, 128).T.astype(np.float32))
        in_maps.append({"qT": qb, "kT": kb, "vb": vb, "ebT": ebT})
    return in_maps


def _gather(results):
    out = np.empty((B, SQ, H, D), dtype=np.float32)
    for core in range(NCORES):
        b = core // (NCORES // B)
        h0 = (core % (NCORES // B)) * NSL
        # device out is (NSL, D, SQ) -> (SQ, NSL, D)
        out[b, :, h0 : h0 + NSL, :] = results[core]["out"].transpose(2, 0, 1)
    return out


def kernel(q, kv, attn_bias, key_padding_mask):
    from concourse.bass_utils import run_bass_kernel_spmd

    nc = _get_nc()
    in_maps = _make_in_maps(q, kv, attn_bias, key_padding_mask)
    res = run_bass_kernel_spmd(nc, in_maps, list(range(NCORES)))
    return _gather(res.results)


# revision 21
# speedup vs baseline: 1.0779x; 1.0623x over previous
"""Fused multi-head cross-attention for Trainium2, SPMD over 8 NeuronCores.

Problem: out = softmax(q @ k^T / sqrt(D) + attn_bias + pad_mask) @ v
  q: (B=4, Sq=2048, H=16, D=128) f32, kv: (B, Sk=2048, 2, H, D) f32,
  attn_bias: (B, Sk) f32, key_padding_mask: (B, Sk) bool -> out (B, Sq, H, D) f32

Sharding: 64 (b, h) slices; core k owns batch k//2, heads (k%2)*8..+8.

v3 design (engine-balanced around the ACT-engine exp roofline, ~291us/rep
HW vs 318.7us baseline; ACT busy ~266us is the modeled floor):
  - Q^T, K^T are pre-transposed on the host (D-major), so device loads are
    linear DMAs (no XBAR transpose).  S^T = K @ Q^T is computed in 512-wide
    PSUM blocks, chunk-pure pairs of banks, so the per-key attn bias
    (+ mask - ln16) fuses into the ACT exponential's per-partition bias
    slot at zero extra engine cost: pp = exp(scale*s + b)/16.  The 1/16
    prescale cancels in the normalize and keeps the fp16 tree < 65504.
  - PV runs TRANSPOSED (out^T = sum_c V_c^T @ P'_c per 512-q block) so all
    PE matmuls have 512-wide moving operands; v1's 256 short 129-wide PV
    matmuls ran at the ~81ns/MM LDWEIGHTS floor and made PE the bottleneck.
  - The softmax denominator: fp16 pairwise tree over pp chunks (DVE 2x),
    then per 512-q block a 1-column ones matmul gives l = sum_k P', a
    custom-DVE reciprocal (f16 out) gives 1/l, a K=1 fp16 ones matmul
    broadcasts it across partitions into PSUM, DVE copies it to SBUF
    (DVE cannot read 2 PSUM operands) and multiplies out^T by it.
  - NO GPSIMD: its tensor ops cost ~25us per [128,2048] op on HW.
  - PSUM: 3x2 banks S^T triple buffer + 1 bank PV accum + 1 shared bank
    psl/rlb = exactly 8.  Triple-buffered S^T is load-bearing: bufs=2
    measured 318.7us vs 291.2us.
"""

import sys

if "/opt/trn_rl_repo" not in sys.path:
    sys.path.insert(0, "/opt/trn_rl_repo")

import numpy as np
import ml_dtypes

B, SQ, SK, H, D = 4, 2048, 2048, 16, 128
NCORES = 8
NSL = H * B // NCORES  # 8 head-slices per core
CK = SK // 128  # 16 sk chunks
NQB = SQ // 512  # 4 q blocks of 512
SCALE = float(1.0 / np.sqrt(np.float32(D)))
EBSCL = 1.0 / 16.0  # prescale on e^bias; cancels in normalize, guards fp16

_CACHE = {}


def _build_nc(nrep=1, lead=4, tree_lag=3, pss_bufs=3, pv_bufs=1):
    """nrep > 1 repeats the whole per-core computation (same inputs/outputs)
    back-to-back; used only for wall-clock timing (device work >> RPC cost)."""
    import concourse.bacc as bacc
    import concourse.tile as tile
    import concourse.mybir as mybir

    f32 = mybir.dt.float32
    bf16 = mybir.dt.bfloat16
    f16 = mybir.dt.float16
    mul_op = mybir.AluOpType.mult
    add_op = mybir.AluOpType.add

    nc = bacc.Bacc("TRN2", target_bir_lowering=False, debug=False)
    qd = nc.dram_tensor("qT", [NSL, D, SQ], bf16, kind="ExternalInput").ap()
    kd = nc.dram_tensor("kT", [NSL, D, SK], bf16, kind="ExternalInput").ap()
    vd = nc.dram_tensor("vb", [NSL, SK, D], bf16, kind="ExternalInput").ap()
    # additive attn_bias + mask - ln16 per key, laid out (sk%128, chunk)
    bd = nc.dram_tensor("ebT", [128, CK], f32, kind="ExternalInput").ap()
    od = nc.dram_tensor("out", [NSL, D, SQ], f32, kind="ExternalOutput").ap()

    GRP = 2  # blocks per S^T group; chunk-pure so the ACT bias is uniform
    NBLK = CK * NQB  # 64 S^T blocks per slice
    groups = [(b0, GRP) for b0 in range(0, NBLK, GRP)]

    # group index after which chunk c is fully exponentiated
    def grp_done(c):
        return 2 * c + 1

    with tile.TileContext(nc) as tc:
        with (
            tc.tile_pool(name="qt", bufs=2) as qt_pool,
            tc.tile_pool(name="kt", bufs=2) as kt_pool,
            tc.tile_pool(name="vp", bufs=3) as vp_pool,
            tc.tile_pool(name="pp", bufs=2) as pp_pool,
            tc.tile_pool(name="swl", bufs=1) as swl_pool,
            tc.tile_pool(name="swsum", bufs=2) as swsum_pool,
            tc.tile_pool(name="bias", bufs=1) as bias_pool,
            tc.tile_pool(name="ot", bufs=2) as ot_pool,
            tc.tile_pool(name="rl", bufs=1) as rl_pool,
            tc.tile_pool(name="rlb", bufs=1) as rlb_pool,
            tc.tile_pool(name="psS", bufs=pss_bufs, space="PSUM") as psS_pool,  # 2 banks
            tc.tile_pool(name="psV", bufs=pv_bufs, space="PSUM") as psV_pool,  # 1 bank
            tc.tile_pool(name="psL", bufs=1, space="PSUM") as psL_pool,  # 1 bank
        ):
            bias_sb = bias_pool.tile([128, CK], f32)
            nc.sync.dma_start(bias_sb[:], bd[:])
            ones_col = bias_pool.tile([128, 1], f16)
            nc.vector.memset(ones_col[:], 1.0)
            ones_row = bias_pool.tile([1, 128], f16)
            nc.vector.memset(ones_row[:], 1.0)
            # pairwise-tree scratch: 9 slots of [128, SQ] f16
            swl = swl_pool.tile([128, 9, SQ], f16)

            slice_tiles = {}

            def load_slice(s):
                qt_t = qt_pool.tile([128, SQ], bf16)
                nc.sync.dma_start(qt_t[:], qd[s])
                kt_t = kt_pool.tile([128, SK], bf16)
                nc.sync.dma_start(kt_t[:], kd[s])
                vp_t = vp_pool.tile([128, CK, D], bf16)
                nc.sync.dma_start(
                    vp_t[:], vd[s].rearrange("(c p) d -> p c d", p=128)
                )
                pp_t = pp_pool.tile([128, CK, SQ], bf16)
                sw_t = swsum_pool.tile([128, SQ], f16)
                slice_tiles[s] = (qt_t, kt_t, vp_t, pp_t, sw_t)

            def s_group(s, gi):
                b0, nblk = groups[gi]
                qt_t, kt_t, _, pp_t, _ = slice_tiles[s]
                c0 = b0 // NQB
                ps = psS_pool.tile([128, GRP * 512], f32, name=f"ps{gi}", tag="ps")
                for j in range(nblk):
                    b = b0 + j
                    c, qcol = divmod(b, NQB)
                    nc.tensor.matmul(
                        ps[:, j * 512 : (j + 1) * 512],
                        lhsT=kt_t[:, c * 128 : (c + 1) * 128],
                        rhs=qt_t[:, qcol * 512 : (qcol + 1) * 512],
                        start=True,
                        stop=True,
                    )
                pp_flat = pp_t.rearrange("p c q -> p (c q)")
                nc.scalar.activation(
                    pp_flat[:, b0 * 512 : (b0 + nblk) * 512],
                    ps[:, 0 : nblk * 512],
                    mybir.ActivationFunctionType.Exp,
                    bias=bias_sb[:, c0 : c0 + 1],
                    scale=SCALE,
                )

            # DVE work tied to S groups: the pairwise fp16 tree summing pp
            # chunks toward swsum (the softmax denominator row).  Returns
            # a dict keyed by group index -> list of emission callables.
            def dve_plan(s):
                _, _, _, pp_t, sw_t = slice_tiles[s]
                plan = {gi: [] for gi in range(len(groups))}

                def tt(out_ap, a_ap, b_ap):
                    def f():
                        nc.vector.tensor_tensor(out_ap, a_ap, b_ap, add_op)
                    return f

                n_g = len(groups)

                def lag(gi):
                    return min(gi + tree_lag, n_g - 1)

                # L0: 8 pair sums of pp chunks -> swl slots 0..7
                for j in range(8):
                    plan[lag(grp_done(2 * j + 1))].append(
                        tt(swl[:, j, :], pp_t[:, 2 * j, :], pp_t[:, 2 * j + 1, :])
                    )
                g_l1 = [lag(grp_done(3)), lag(grp_done(7)), lag(grp_done(11)),
                        lag(grp_done(15))]
                # L1: slots (0,1)->8, (2,3)->0', reuse via distinct slots
                plan[g_l1[0]].append(tt(swl[:, 8, :], swl[:, 0, :], swl[:, 1, :]))
                plan[g_l1[1]].append(tt(swl[:, 0, :], swl[:, 2, :], swl[:, 3, :]))
                plan[g_l1[2]].append(tt(swl[:, 1, :], swl[:, 4, :], swl[:, 5, :]))
                plan[g_l1[3]].append(tt(swl[:, 2, :], swl[:, 6, :], swl[:, 7, :]))
                # L2 + L3 -> swsum
                plan[g_l1[3]].append(tt(swl[:, 3, :], swl[:, 8, :], swl[:, 0, :]))
                plan[g_l1[3]].append(tt(swl[:, 4, :], swl[:, 1, :], swl[:, 2, :]))
                plan[g_l1[3]].append(tt(sw_t[:], swl[:, 3, :], swl[:, 4, :]))
                return plan

            # Output phase for slice s (runs one step later): per 512-q block:
            # l row matmul, 1/l, partition-broadcast matmul, 16 PV^T matmuls,
            # normalize, store.
            def out_phase_items(s):
                _, _, vp_t, pp_t, sw_t = slice_tiles[s]
                items = []
                state = {}

                def psl(j):
                    def f():
                        ps_l = psL_pool.tile([1, 512], f32, name=f"psl{j}", tag="psl")
                        state["psl"] = ps_l
                        nc.tensor.matmul(
                            ps_l[:],
                            lhsT=ones_col[:],
                            rhs=sw_t[:, j * 512 : (j + 1) * 512],
                            start=True,
                            stop=True,
                        )
                    return f

                def rb(j):
                    def f():
                        from concourse.dve_ops import (
                            RECIP_APPROX_FAST_CONSTS,
                            RECIPROCAL_APPROX_FAST,
                        )
                        rc = RECIP_APPROX_FAST_CONSTS
                        rl_h = rl_pool.tile([1, 512], f16, name=f"rlh{j}", tag="rl")
                        nc.vector._custom_dve(
                            RECIPROCAL_APPROX_FAST,
                            out=rl_h[:],
                            in0=state["psl"][:],
                            s0=rc["s0"],
                            s1=rc["s1"],
                            imm2=rc["imm2"],
                        )
                        rlb = psL_pool.tile([128, 512], f32, name=f"rlb{j}", tag="psl")
                        nc.tensor.matmul(
                            rlb[:], lhsT=ones_row[:], rhs=rl_h[:],
                            start=True, stop=True,
                        )
                        rlb_sb = rlb_pool.tile(
                            [128, 512], f32, name=f"rlbs{j}", tag="rlbs"
                        )
                        state["rlb"] = rlb_sb
                        nc.vector.tensor_copy(rlb_sb[:], rlb[:])
                    return f

                def pv(j, c0, c1):
                    def f():
                        if c0 == 0:
                            state["pv"] = psV_pool.tile([128, 512], f32, name=f"pv{j}", tag="pv")
                        po = state["pv"]
                        for c in range(c0, c1):
                            nc.tensor.matmul(
                                po[:],
                                lhsT=vp_t[:, c, :],
                                rhs=pp_t[:, c, j * 512 : (j + 1) * 512],
                                start=(c == 0),
                                stop=(c == CK - 1),
                            )
                    return f

                def fin(j):
                    def f():
                        ot = ot_pool.tile([128, 512], f32, name=f"ot{j}", tag="ot")
                        nc.vector.tensor_tensor(
                            ot[:], state["pv"][:], state["rlb"][:], mul_op
                        )
                        nc.sync.dma_start(od[s, :, j * 512 : (j + 1) * 512], ot[:])
                    return f

                for j in range(NQB):
                    items.append(psl(j))
                    items.append(pv(j, 0, 2))
                    items.append(rb(j))
                    for c0 in range(2, CK, 2):
                        items.append(pv(j, c0, c0 + 2))
                    items.append(fin(j))
                return items

            NV = NSL * nrep  # total virtual slices

            def emit_step(v):
                s = v % NSL
                do_s = v < NV
                if v + 1 < NV:
                    load_slice((v + 1) % NSL)
                oq = out_phase_items((v - 1) % NSL) if v > 0 else []
                if not do_s:
                    for f in oq:
                        f()
                    return
                plan = dve_plan(s)
                n_g = len(groups)
                oi = 0
                acc = 0.0
                ratio = len(oq) / max(1, n_g - lead - 3)
                for gi in range(n_g):
                    s_group(s, gi)
                    for f in plan[gi]:
                        f()
                    if gi >= lead:
                        acc += ratio
                        while acc >= 1.0 and oi < len(oq):
                            oq[oi]()
                            oi += 1
                            acc -= 1.0
                while oi < len(oq):
                    oq[oi]()
                    oi += 1

            load_slice(0)
            for v in range(NV + 1):
                emit_step(v)

    nc.compile()
    return nc


def _get_nc():
    if "nc" not in _CACHE:
        _CACHE["nc"] = _build_nc()
    return _CACHE["nc"]


def _make_in_maps(q, kv, attn_bias, key_padding_mask):
    q = np.asarray(q)
    kv = np.asarray(kv)
    attn_bias = np.asarray(attn_bias, dtype=np.float32)
    key_padding_mask = np.asarray(key_padding_mask)

    biasp = attn_bias + np.where(key_padding_mask, 0.0, -1e30).astype(np.float32)
    ebias = (biasp + np.log(EBSCL)).astype(np.float32)  # additive; masked -> -inf
    bf16 = ml_dtypes.bfloat16

    in_maps = []
    for core in range(NCORES):
        b = core // (NCORES // B)
        h0 = (core % (NCORES // B)) * NSL
        # (Sq, NSL, D) -> (NSL, D, Sq) pre-transposed for D-major loads
        qb = np.ascontiguousarray(
            q[b, :, h0 : h0 + NSL, :].transpose(1, 2, 0)
        ).astype(bf16)
        kb = np.ascontiguousarray(
            kv[b, :, 0, h0 : h0 + NSL, :].transpose(1, 2, 0)
        ).astype(bf16)
        vb = np.ascontiguousarray(
            kv[b, :, 1, h0 : h0 + NSL, :].transpose(1, 0, 2)
        ).astype(bf16)
        ebT = np.ascontiguousarray(ebias[b].reshape(CK, 128).T.astype(np.float32))
        in_maps.append({"qT": qb, "kT": kb, "vb": vb, "ebT": ebT})
    return in_maps


def _gather(results):
    out = np.empty((B, SQ, H, D), dtype=np.float32)
    for core in range(NCORES):
        b = core // (NCORES // B)
        h0 = (core % (NCORES // B)) * NSL
        # device out is (NSL, D, SQ) -> (SQ, NSL, D)
        out[b, :, h0 : h0 + NSL, :] = results[core]["out"].transpose(2, 0, 1)
    return out


def kernel(q, kv, attn_bias, key_padding_mask):
    from concourse.bass_utils import run_bass_kernel_spmd

    nc = _get_nc()
    in_maps = _make_in_maps(q, kv, attn_bias, key_padding_mask)
    res = run_bass_kernel_spmd(nc, in_maps, list(range(NCORES)))
    return _gather(res.results)
